# revision 20
# baseline (speedup 1.0000x reference)
"""CorrelationHead Trainium2 kernel (v5).

Math: SpatialCorrelationSampler(patch=16, dil=2) on 7x7 maps -> corr features
collapse to the per-RoI Gram matrix G[ij, kl] = sum_c x1[b,c,ij] x2[b,c,kl],
valid only when positions (i,j) and (k,l) have matching parity (625 valid
pairs of 2401). fc1(corr) = sum over valid (ij,kl) of G * W1eff[kl,ij,:].

Device plan (per core, 128 RoIs, pure data-parallel over 8 cores):
- x1 [128c, 2 half, 128b*49] and x2 [128c, 2 half, 128b*53] (x2's position
  columns parity-permuted and padded to 53 so each parity class sits inside
  one 32-partition PE quadrant); 8 chunk DMA pairs on the SP ring first.
- Gram on PE: psG[kl53, b, ij] = x2^T x1 per RoI (2 bf16 matmuls K=128).
  16 groups of 8 RoIs through a 3-bank PSUM ring.
- Batched evictions (f32->bf16, 8 RoIs each) alternate ACT/DVE into
  gsa[53 kl, 49 ij, 128 b].
- Valid pairs packed to K=128 tiles by 59 masked-selection matmuls on PE
  (host-precomputed 32x32 selection blocks; every operand 32-aligned),
  keeping PE continuously busy; ACT/DVE evict psPacked -> packed[128,6,128].
- fc1/fc2 computed TRANSPOSED (out[rep, b]): no PE transposes or copybacks.
  fc1T = 48 matmuls vs the 98 K=49 matmuls of the naive layout.
- Weights ride the Pool/SWDGE ring (own descriptor path), gated behind x.
"""

import os
import numpy as np

import concourse.bass as bass
import concourse.mybir as mybir
from concourse.bass_utils import run_bass_kernel_spmd

# ---------------------------------------------------------------- constants
P = 16
H = 7
HW = H * H            # 49
KL = 56               # padded kl rows (classes at 0, 16, 32, 44)
C = 256
B = 1024
REP = 1024
N_CORES = 8
BL = B // N_CORES     # 128 RoIs per core

CH = 16               # RoIs per x chunk DMA pair
NCHUNK = BL // CH     # 8
NG = 8                # RoIs per PSUM gram group
NGROUP = BL // NG     # 16
GPC = CH // NG        # groups per chunk = 2
PSG_RING = 3

F32 = mybir.dt.float32
BF16 = mybir.dt.bfloat16
F8 = mybir.dt.float8e4

XDT_MODE = os.environ.get("CORR_XDT", "bf16")   # "bf16" | "fp8"

LAST_EXEC_NS = None
_CACHE = {}


# ------------------------------------------------------- parity permutation
def _parity_perm():
    """Positions grouped by (i%2, j%2): ee(16), eo(12), oe(12), oo(9)."""
    cls = {(0, 0): [], (0, 1): [], (1, 0): [], (1, 1): []}
    for i in range(H):
        for j in range(H):
            cls[(i % 2, j % 2)].append(i * H + j)
    return cls[(0, 0)] + cls[(0, 1)] + cls[(1, 0)] + cls[(1, 1)]


PIJ = _parity_perm()

# pack chunks: (ij0, nij, kl0, nkl, tile, row0) in parity-ordered indices.
# class ij/kl (unpadded) ranges: ee [0,16) eo [16,28) oe [28,40) oo [40,49)
CHUNKS = [
    (0, 8, 0, 16, 0, 0),
    (8, 8, 0, 16, 1, 0),
    (16, 10, 16, 12, 2, 0),
    (26, 2, 16, 12, 3, 0),
    (28, 8, 28, 12, 3, 24),
    (36, 4, 28, 12, 4, 0),
    (40, 8, 40, 9, 4, 48),
    (48, 1, 40, 9, 5, 0),
]
NTILE = 6
C0P = {0: 0, 16: 16, 28: 32, 40: 44}   # class kl base -> padded gsa row base

# Selection-matmul plan: one matmul per (tile, 32-row window, ij segment
# piece). Each entry: (tile, A, W, ksz, blk, ij, start, stop). Fully-empty
# windows get a zero block (blk = NSEL - 1).
_segs = {}
for (_ij0, _nij, _kl0, _nkl, _t, _r0) in CHUNKS:
    for _a in range(_nij):
        _segs.setdefault(_t, []).append(
            (_r0 + _a * _nkl, _nkl, _ij0 + _a, C0P[_kl0])
        )
SELMM = []        # (tile, blk, ij, start, stop)
_SELBLOCKS = []   # (blk, r0, sc, c0p) for host selmat construction
_blk = 0
for _t in range(NTILE):
    _lst = sorted(_segs.get(_t, []))
    for _n, (_sr0, _sc, _sij, _c0p) in enumerate(_lst):
        SELMM.append((_t, _blk, _sij, _n == 0, _n == len(_lst) - 1))
        _SELBLOCKS.append((_blk, _sr0, _sc, _c0p))
        _blk += 1
NSEL = _blk

# smalls layout within sms[1, SM_LEN]
SM_B1 = 0
SM_B2 = 1024
SM_B3 = 2048
SM_ONES = 2052
SM_LEN = 2180


# ---------------------------------------------------------------- host prep
def _w1_packed(W1, np_dt):
    """[128 rows, NTILE, REP]; row = (tile, row0+a*nkl+kk) per CHUNKS; pad 0."""
    w = np.zeros((128, NTILE, REP), dtype=np.float32)
    for (ij0, nij, kl0, nkl, t, r0) in CHUNKS:
        for a in range(nij):
            ij_o = PIJ[ij0 + a]
            i, j = divmod(ij_o, H)
            for kk in range(nkl):
                kl_o = PIJ[kl0 + kk]
                k, l = divmod(kl_o, H)
                ph = (k - i) // 2 + 7
                pw = (l - j) // 2 + 7
                f = (ph * P + pw) * HW + ij_o
                w[r0 + a * nkl + kk, t, :] = W1[:, f]
    return w.astype(np_dt)


def _selmat(np_dt):
    """[64, NSEL*128] masked row-placement blocks for the pack matmuls."""
    sm = np.zeros((64, NSEL * 128), dtype=np.float32)
    for (blk, r0, sc, c0p) in _SELBLOCKS:
        for s in range(sc):
            sm[c0p + s, blk * 128 + r0 + s] = 1.0
    return sm.astype(np_dt)


def _x_pair(p1, p2, np_dt):
    """-> x1 [128, 2, BL*49], x2 padded [128, 2, BL*53], parity-permuted."""
    a1 = p1[:, :, PIJ]
    a1 = a1.reshape(BL, 2, 128, HW).transpose(2, 1, 0, 3)
    x1 = np.ascontiguousarray(a1.reshape(128, 2, BL * HW)).astype(np_dt)
    a2 = np.zeros((BL, C, KL), dtype=np.float32)
    a2[:, :, 0:28] = p2[:, :, PIJ[0:28]]
    a2[:, :, 32:53] = p2[:, :, PIJ[28:]]
    a2 = a2.reshape(BL, 2, 128, KL).transpose(2, 1, 0, 3)
    x2 = np.ascontiguousarray(a2.reshape(128, 2, BL * KL)).astype(np_dt)
    return x1, x2


# ---------------------------------------------------------------- device IR
def _build(xdt):
    nc = bass.Bass()
    fp8 = xdt == F8

    x1h = nc.dram_tensor("x1h", [128, 2, BL * HW], xdt, kind="ExternalInput")
    x2h = nc.dram_tensor("x2h", [128, 2, BL * KL], xdt, kind="ExternalInput")
    w1h = nc.dram_tensor("w1h", [128, NTILE * REP], BF16, kind="ExternalInput")
    w2h = nc.dram_tensor("w2h", [128, 8 * REP], BF16, kind="ExternalInput")
    w3h = nc.dram_tensor("w3h", [128, 8 * 4], BF16, kind="ExternalInput")
    smh = nc.dram_tensor("smh", [1, SM_LEN], BF16, kind="ExternalInput")
    selh = nc.dram_tensor("selh", [64, NSEL * 128], BF16, kind="ExternalInput")
    zbh = nc.dram_tensor("zbh", [128, 1], F32, kind="ExternalInput")
    outh = nc.dram_tensor("outh", [128, 4], F32, kind="ExternalOutput")

    from contextlib import ExitStack

    with ExitStack() as ctx:
        sb = lambda name, shape, d: ctx.enter_context(nc.sbuf_tensor(name, shape, d))
        ps = lambda name, shape, d: ctx.enter_context(nc.psum_tensor(name, shape, d))
        sem = lambda name: ctx.enter_context(nc.semaphore(name))

        xs1 = sb("xs1", [128, 2, BL * HW], xdt)
        xs2 = sb("xs2", [128, 2, BL * KL], xdt)
        gsa = sb("gsa", [KL, HW, 128], BF16)
        packed = sb("packed", [128, NTILE, 128], BF16)
        w1p = sb("w1p", [128, NTILE, REP], BF16)
        w2s = sb("w2s", [128, 8, REP], BF16)
        w3s = sb("w3s", [128, 8, 4], BF16)
        sms = sb("sms", [1, SM_LEN], BF16)
        sels = sb("sels", [64, NSEL * 128], BF16)
        zbs = sb("zbs", [128, 1], F32)
        r1T = sb("r1T", [128, 8, 128], BF16)
        r2T = sb("r2T", [128, 8, 128], BF16)
        outs = sb("outs", [128, 4], F32)

        psG = [ps(f"psG{q}", [KL, NG, HW], F32) for q in range(PSG_RING)]
        psPA = ps("psPA", [128, 4, 128], F32)
        psPB = ps("psPB", [128, 2, 128], F32)
        psFA = ps("psFA", [128, 4, 128], F32)
        psFB = ps("psFB", [128, 4, 128], F32)
        psO = ps("psO", [128, 4], F32)

        s_xc = [sem(f"s_xc{i}") for i in range(NCHUNK)]
        s_ws = sem("s_ws")
        s_w1 = sem("s_w1")
        s_w2 = sem("s_w2")
        s_g = sem("s_g")
        s_ea = sem("s_ea")
        s_ed = sem("s_ed")
        s_sa = sem("s_sa")
        s_sb = sem("s_sb")
        s_pa = sem("s_pa")
        s_pb = sem("s_pb")
        s_f1 = sem("s_f1")
        s_r1a = sem("s_r1a")
        s_r1b = sem("s_r1b")
        s_f2 = sem("s_f2")
        s_r2a = sem("s_r2a")
        s_r2b = sem("s_r2b")
        s_f3 = sem("s_f3")
        s_oe = sem("s_oe")
        s_o = sem("s_o")
        block = ctx.enter_context(nc.Block())

        ones_ap = sms[0:1, SM_ONES : SM_ONES + 128]

        # -------- SP: x chunk DMAs, final output DMA
        @block.sync
        def _(sp):
            for ch in range(NCHUNK):
                sp.dma_start(
                    xs1[:, :, ch * CH * HW : (ch + 1) * CH * HW],
                    x1h[:, :, ch * CH * HW : (ch + 1) * CH * HW],
                ).then_inc(s_xc[ch], 16)
                sp.dma_start(
                    xs2[:, :, ch * CH * KL : (ch + 1) * CH * KL],
                    x2h[:, :, ch * CH * KL : (ch + 1) * CH * KL],
                ).then_inc(s_xc[ch], 16)
            sp.wait_ge(s_oe, 1)
            sp.dma_start(outh[:, :], outs[:, :]).then_inc(s_o, 16)
            sp.wait_ge(s_o, 16)

        # -------- Pool/SWDGE: smalls at t=0, big weights gated behind x
        @block.gpsimd
        def _(gp):
            gp.dma_start(sms[:, :], smh[:, :]).then_inc(s_ws, 16)
            gp.dma_start(zbs[:, :], zbh[:, :]).then_inc(s_ws, 16)
            gp.wait_ge(s_xc[3], 32)  # 4 x chunks transferred
            gp.dma_start(sels[:, :], selh[:, :]).then_inc(s_ws, 16)
            gp.dma_start(w1p[:, 0:3, :], w1h[:, 0 : 3 * REP]).then_inc(s_w1, 16)
            gp.dma_start(w1p[:, 3:6, :], w1h[:, 3 * REP : 6 * REP]).then_inc(s_w1, 16)
            for q in range(4):
                gp.dma_start(
                    w2s[:, 2 * q : 2 * q + 2, :],
                    w2h[:, 2 * q * REP : (2 * q + 2) * REP],
                ).then_inc(s_w2, 16)
            gp.dma_start(w3s[:, :, :], w3h[:, :]).then_inc(s_w2, 16)

        # -------- PE: Gram, pack (selection matmuls), fc1T, fc2T, fc3
        @block.tensor
        def _(pe):
            for ch in range(NCHUNK):
                pe.wait_ge(s_xc[ch], 32)
                for g in range(GPC):
                    gi = ch * GPC + g
                    q = gi % PSG_RING
                    if gi >= PSG_RING:
                        pg = gi - PSG_RING
                        pe.wait_ge(s_ed if pg % 2 else s_ea, pg // 2 + 1)
                    for bb in range(NG):
                        lb = gi * NG + bb
                        s1 = slice(lb * HW, (lb + 1) * HW)
                        s2 = slice(lb * KL, (lb + 1) * KL)
                        if fp8:
                            mm = pe.matmul(
                                psG[q][:, bb, :],
                                xs2[:, :, s2],
                                xs1[:, :, s1],
                                start=True,
                                stop=True,
                                perf_mode=mybir.MatmulPerfMode.DoubleRow,
                            )
                        else:
                            for t in range(2):
                                mm = pe.matmul(
                                    psG[q][:, bb, :],
                                    xs2[:, t, s2],
                                    xs1[:, t, s1],
                                    start=(t == 0),
                                    stop=(t == 1),
                                )
                    mm.then_inc(s_g, 1)

            # pack: masked-selection matmuls gsa windows -> psPacked windows
            pe.wait_ge(s_ea, NGROUP // 2)
            pe.wait_ge(s_ed, NGROUP // 2)
            pe.wait_ge(s_ws, 48)
            last_a = max(i for i, e in enumerate(SELMM) if e[0] < 4)
            for idx, (t, blk, ij, st, sp_) in enumerate(SELMM):
                out = psPA[:, t, :] if t < 4 else psPB[:, t - 4, :]
                mm = pe.matmul(
                    out,
                    sels[0:KL, blk * 128 : blk * 128 + 128],
                    gsa[:, ij, :],
                    start=st,
                    stop=sp_,
                )
                if idx == last_a:
                    mm.then_inc(s_sa, 1)
                elif idx == len(SELMM) - 1:
                    mm.then_inc(s_sb, 1)

            # fc1T: psF1T[rep, b] = sum_t w1p_t^T packed_t (+ b1)
            pe.wait_ge(s_pa, 1)
            pe.wait_ge(s_pb, 1)
            pe.wait_ge(s_w1, 32)
            for bank, psF in ((0, psFA), (1, psFB)):
                for mi in range(4):
                    m = bank * 4 + mi
                    for t in range(NTILE):
                        pe.matmul(
                            psF[:, mi, :],
                            w1p[:, t, m * 128 : (m + 1) * 128],
                            packed[:, t, :],
                            start=(t == 0),
                            stop=False,
                        )
                    mm = pe.matmul(
                        psF[:, mi, :],
                        sms[0:1, SM_B1 + m * 128 : SM_B1 + (m + 1) * 128],
                        ones_ap,
                        start=False,
                        stop=True,
                    )
                mm.then_inc(s_f1, 1)

            # fc2T: psF2T[rep2, b] = sum_k w2_k^T r1T_k (+ b2)
            pe.wait_ge(s_r1a, 1)
            pe.wait_ge(s_r1b, 1)
            pe.wait_ge(s_w2, 80)
            for bank, psF in ((0, psFA), (1, psFB)):
                for mi in range(4):
                    m = bank * 4 + mi
                    for k in range(8):
                        pe.matmul(
                            psF[:, mi, :],
                            w2s[:, k, m * 128 : (m + 1) * 128],
                            r1T[:, k, :],
                            start=(k == 0),
                            stop=False,
                        )
                    mm = pe.matmul(
                        psF[:, mi, :],
                        sms[0:1, SM_B2 + m * 128 : SM_B2 + (m + 1) * 128],
                        ones_ap,
                        start=False,
                        stop=True,
                    )
                mm.then_inc(s_f2, 1)

            # fc3: psO[b, 4] = sum_k r2T_k^T w3_k (+ b3)
            pe.wait_ge(s_r2a, 1)
            pe.wait_ge(s_r2b, 1)
            for k in range(8):
                pe.matmul(
                    psO[:, :],
                    r2T[:, k, :],
                    w3s[:, k, :],
                    start=(k == 0),
                    stop=False,
                )
            pe.matmul(
                psO[:, :],
                ones_ap,
                sms[0:1, SM_B3 : SM_B3 + 4],
                start=False,
                stop=True,
            ).then_inc(s_f3, 1)

        # -------- ACT: even-group evictions, bank-A evict/ReLUs, final copy
        @block.scalar
        def _(act):
            for gi in range(0, NGROUP, 2):
                q = gi % PSG_RING
                act.wait_ge(s_g, gi + 1)
                act.activation(
                    gsa[:, :, gi * NG : (gi + 1) * NG],
                    psG[q][:, :, :].rearrange("p b i -> p i b"),
                    mybir.ActivationFunctionType.Copy,
                ).then_inc(s_ea, 1)
            act.wait_ge(s_ws, 48)
            act.wait_ge(s_sa, 1)
            act.activation(
                packed[:, 0:4, :], psPA[:, :, :],
                mybir.ActivationFunctionType.Copy,
            ).then_inc(s_pa, 1)
            act.wait_ge(s_f1, 1)
            act.activation(
                r1T[:, 0:4, :], psFA[:, :, :],
                mybir.ActivationFunctionType.Relu, bias=zbs[:, :],
            ).then_inc(s_r1a, 1)
            act.wait_ge(s_f2, 1)
            act.activation(
                r2T[:, 0:4, :], psFA[:, :, :],
                mybir.ActivationFunctionType.Relu, bias=zbs[:, :],
            ).then_inc(s_r2a, 1)
            act.wait_ge(s_f3, 1)
            act.activation(
                outs[:, :], psO[:, :], mybir.ActivationFunctionType.Copy
            ).then_inc(s_oe, 1)

        # -------- DVE: odd-group evictions, bank-B evict/ReLUs
        @block.vector
        def _(dve):
            for gi in range(1, NGROUP, 2):
                q = gi % PSG_RING
                dve.wait_ge(s_g, gi + 1)
                dve.tensor_copy(
                    gsa[:, :, gi * NG : (gi + 1) * NG],
                    psG[q][:, :, :].rearrange("p b i -> p i b"),
                ).then_inc(s_ed, 1)
            dve.wait_ge(s_sb, 1)
            dve.tensor_copy(packed[:, 4:6, :], psPB[:, :, :]).then_inc(s_pb, 1)
            dve.wait_ge(s_f1, 2)
            dve.tensor_scalar_max(r1T[:, 4:8, :], psFB[:, :, :], 0.0).then_inc(
                s_r1b, 1
            )
            dve.wait_ge(s_f2, 2)
            dve.tensor_scalar_max(r2T[:, 4:8, :], psFB[:, :, :], 0.0).then_inc(
                s_r2b, 1
            )

    return nc


def _get_nc(xdt):
    key = ("nc", str(xdt))
    if key not in _CACHE:
        _CACHE[key] = _build(xdt)
    return _CACHE[key]


# ---------------------------------------------------------------- entry
def kernel(patch1, patch2, W1, b1, W2, b2, W3, b3):
    global LAST_EXEC_NS
    import ml_dtypes

    xdt = F8 if XDT_MODE == "fp8" else BF16
    np_x = ml_dtypes.float8_e4m3fn if XDT_MODE == "fp8" else ml_dtypes.bfloat16
    np_m = ml_dtypes.bfloat16

    patch1 = np.asarray(patch1, dtype=np.float32).reshape(B, C, HW)
    patch2 = np.asarray(patch2, dtype=np.float32).reshape(B, C, HW)
    W1 = np.asarray(W1, dtype=np.float32)
    W2 = np.asarray(W2, dtype=np.float32)
    W3 = np.asarray(W3, dtype=np.float32)
    b1 = np.asarray(b1, dtype=np.float32)
    b2 = np.asarray(b2, dtype=np.float32)
    b3 = np.asarray(b3, dtype=np.float32)

    w1e = _w1_packed(W1, np_m).reshape(128, NTILE * REP)
    w2e = np.ascontiguousarray(
        W2.T.reshape(8, 128, REP).transpose(1, 0, 2).reshape(128, 8 * REP)
    ).astype(np_m)
    w3e = np.ascontiguousarray(
        W3.T.reshape(8, 128, 4).transpose(1, 0, 2).reshape(128, 32)
    ).astype(np_m)
    sm = np.zeros((1, SM_LEN), dtype=np.float32)
    sm[0, SM_B1 : SM_B1 + REP] = b1
    sm[0, SM_B2 : SM_B2 + REP] = b2
    sm[0, SM_B3 : SM_B3 + 4] = b3
    sm[0, SM_ONES : SM_ONES + 128] = 1.0

    shared = {
        "w1h": w1e,
        "w2h": w2e,
        "w3h": w3e,
        "smh": sm.astype(np_m),
        "selh": _selmat(np_m),
        "zbh": np.zeros((128, 1), dtype=np.float32),
    }

    in_maps = []
    for i in range(N_CORES):
        sl = slice(i * BL, (i + 1) * BL)
        x1, x2 = _x_pair(patch1[sl], patch2[sl], np_x)
        in_maps.append({"x1h": x1, "x2h": x2, **shared})

    nc = _get_nc(xdt)
    trace = os.environ.get("CORR_TRACE", "0") == "1"
    res = run_bass_kernel_spmd(nc, in_maps, list(range(N_CORES)), trace=trace)
    LAST_EXEC_NS = res.exec_time_ns

    out = np.concatenate(
        [res.results[i]["outh"] for i in range(N_CORES)], axis=0
    ).astype(np.float32)
    return out
